# revision 1
# baseline (speedup 1.0000x reference)
"""CTC loss kernel for Trainium2 (8 NeuronCores, data-parallel over batch).

Strategy
--------
reference computes:  lp = log_softmax(y_pred); CTC forward DP over the
blank-extended label sequence in log space; loss = mean(nll / S).

Device work (per core, 8 of 64 samples):
  1. Stream the [8, 256, 4000] f32 shard once and compute
     Z[n, t] = sum_v exp(x[n, t, v])  (ACT engine, exp + accumulate).
     No max-subtraction is needed: inputs are O(1) so exp is safe, and
     log(sum(exp)) is exact enough in f32.
  2. CTC forward DP in *probability* domain on raw gathered values
     G[n, t, s] = exp(x[n, t, ext[n, s]]) (the softmax normalizer is
     folded out of the recurrence):
        a_t[l] = (a[l] + a[l-1] + skip[l] * a[l-2]) * G_t[l]
     with periodic renormalization by the state-sum to stay in f32
     range; the log of every normalizer is accumulated so
        nll[n] = sum_t log Z[n,t] - log(aT[L-1]+aT[L-2]) - sum_k log s_k.
     The 3-tap windowed sum runs as ONE hand-authored custom DVE op
     (CTC_FIR3_ANT) using the datapath's element-feedback delay chains,
     so a DP step is 2 DVE instructions instead of 4.
  3. Small epilogue: Ln + fused accumulations + one tiny matmul for the
     per-sample partition-group sum of log Z; final [8,1] nll DMA'd out.

Host work: shard batch across cores, gather G via take_along_axis
(tiny, ~2% of the data), build the skip mask / selection constants,
and average the 64 per-sample nll values.

Layout notes: alpha state lives at columns [2:67] of a [8,67] tile
(l -> col l+2). G's per-t stride is 67 with exp(-1e30)=0 in the two
lead columns, so the G-multiply re-zeroes the alpha guard columns every
step — that is what neutralizes the custom op's stale element-feedback
at each instruction boundary.
"""

import numpy as np

import concourse.bass as bass
import concourse.dve_ops as dve_ops
import concourse.tile as tile
from concourse import bacc, mybir
from concourse.bass_utils import run_bass_kernel_spmd
from concourse.dve_spec import Spec, Src0, Src1
from concourse.dve_uop import (
    DISABLE,
    ENABLE,
    AluInp,
    AluOp,
    DelayInp,
    DveOpSpec,
    InpSel,
    OutPath,
    OutSel,
    Trigger,
    UopConfig,
    UopDpConfig,
)

F32 = mybir.dt.float32
AF = mybir.ActivationFunctionType
AX = mybir.AxisListType

# Problem shapes (hardcoded per the harness contract).
N, T, V = 64, 256, 4000
S = 32
L = 2 * S + 1            # 65 extended labels
N_CORES = 8
NPC = N // N_CORES       # 8 samples per core
TPB = 128 // NPC         # 16 time steps per 128-partition stream tile
NT = T // TPB            # 16 stream tiles
RENORM = 12              # renormalize the DP state every RENORM steps
RENORM_STEPS = [t for t in range(1, T) if t % RENORM == RENORM - 1]
NRN = len(RENORM_STEPS)  # recorded normalizers
LP = L + 2               # per-t stride of G: [0, 0, g_0..g_64]
NEGPAD = -1e30           # raw pad value; exp -> exactly 0

_CACHE = {}

# --------------------------------------------------------------------------
# Custom DVE op: out[k] = in0[k] + fb1(in0)[k] + in1[k] * fb2(in0)[k]
# where fbN(in0)[k] = in0[k-N] via element-feedback delay-chain loads
# (DelayInp.CURR_ALU_OUT latches the block's previous-element result).
# Validated on hardware: exact match vs the numpy model.
# --------------------------------------------------------------------------

FIR3_NAME = "CTC_FIR3_ANT"


def _fir3_ref(in0, in1, c0, c1, c2):
    a = np.asarray(in0, np.float32)
    m = np.asarray(in1, np.float32)
    p1 = np.zeros_like(a)
    p1[:, 1:] = a[:, :-1]
    p2 = np.zeros_like(a)
    p2[:, 2:] = a[:, :-2]
    return a + p1 + m * p2


def _build_fir3_uops():
    blocks = [UopDpConfig() for _ in range(8)]

    def passthrough(b, chains):
        for c in chains:
            b.delay[c] = DelayInp.PREV_DELAY
            b.delay_enable[c] = ENABLE

    # chains 0/1 carry the Src0/Src1 streams (loaded at the input stage).
    # b0: flop0 = s0[k]; chain2 <- own flop (= s0[k-1] for the next element)
    blocks[0].enable_alu(AluOp.BYPASS, AluInp.PREV_DELAY_0)
    passthrough(blocks[0], (0, 1))
    blocks[0].delay[2] = DelayInp.CURR_ALU_OUT
    blocks[0].delay_enable[2] = ENABLE
    # b1: flop1 = s0[k-1]; chain3 <- own flop (= s0[k-2])
    blocks[1].enable_alu(AluOp.BYPASS, AluInp.PREV_DELAY_2)
    passthrough(blocks[1], (0, 1, 2))
    blocks[1].delay[3] = DelayInp.CURR_ALU_OUT
    blocks[1].delay_enable[3] = ENABLE
    # b2: flop2 = m[k] * s0[k-2]
    blocks[2].enable_alu(AluOp.MULTIPLY, AluInp.PREV_DELAY_3, AluInp.PREV_DELAY_1)
    passthrough(blocks[2], (0, 2))
    # b3: flop3 = flop2 + s0[k]
    blocks[3].enable_alu(AluOp.ADD, AluInp.PREV_ALU_OUT, AluInp.PREV_DELAY_0)
    passthrough(blocks[3], (2,))
    # b4: flop4 = flop3 + s0[k-1]
    blocks[4].enable_alu(AluOp.ADD, AluInp.PREV_ALU_OUT, AluInp.PREV_DELAY_2)
    # b5-7: carry result to the write stage
    for j in range(5, 8):
        blocks[j].pass_through_alu()

    n_inp = len(UopConfig().inp)
    inp = [InpSel.ZERO] * n_inp
    inp_enable = [DISABLE] * n_inp
    inp[1] = InpSel.SRC_0
    inp_enable[1] = ENABLE
    inp[2] = InpSel.SRC_1
    inp_enable[2] = ENABLE

    out = {p: OutSel.ALU_OUT for p in OutPath}
    out_enable = {p: DISABLE for p in OutPath}
    out_enable[OutPath.WR0_LO] = ENABLE

    return [
        UopConfig(
            inp=inp,
            inp_enable=inp_enable,
            out=out,
            out_enable=out_enable,
            require_inp0=ENABLE,
            require_inp1=ENABLE,
            trigger=(Trigger.SRC_TENSOR_DONE, Trigger.NONE, Trigger.NONE),
            next_uop=(0, 0, 0),
            datapath_config=blocks,
        )
    ]


class _HandAuthoredDveOp:
    """Duck-typed DveOp whose compile() is served from the compile cache."""

    def __init__(self, name, spec_obj, dvespec):
        self.name = name
        self.spec = spec_obj
        self.subdim = False
        self.perf_en = {}
        self._dvespec = dvespec

    def compile(self, ver):
        return self._dvespec


def _register_fir3():
    if FIR3_NAME in dve_ops._SUB_OPCODE_FOR_NAME:
        return next(o for o in dve_ops.OPS if o.name == FIR3_NAME)
    dvespec = DveOpSpec(
        name=FIR3_NAME, uops=_build_fir3_uops(), rd1_en=True, opcode=None
    )
    spec_obj = Spec(body=Src0 + Src1, reference=_fir3_ref)  # body unused
    op = _HandAuthoredDveOp(FIR3_NAME, spec_obj, dvespec)
    row = dve_ops._CUSTOM_DVE_ROW_BASE + len(dve_ops.OPS)
    assert row < 0x20
    dve_ops.OPS.append(op)
    dve_ops._SUB_OPCODE_FOR_NAME[FIR3_NAME] = row
    dve_ops.CUSTOM_DVE_SPECS[FIR3_NAME] = spec_obj
    dvespec.opcode = row
    for ver in ("v3", "v4"):
        dve_ops._COMPILE_CACHE[(FIR3_NAME, ver)] = dvespec
    return op


# --------------------------------------------------------------------------


def _build_program():
    """Build + compile the single SPMD program shared by all 8 cores."""
    fir3 = _register_fir3()
    nc = bacc.Bacc(
        "TRN2",
        target_bir_lowering=False,
        debug=False,
        enable_asserts=False,
        num_devices=1,
    )
    x = nc.dram_tensor("x", [NPC, T, V], F32, kind="ExternalInput").ap()
    g = nc.dram_tensor("g", [NPC, T * LP], F32, kind="ExternalInput").ap()
    skip = nc.dram_tensor("skip", [NPC, LP], F32, kind="ExternalInput").ap()
    sel = nc.dram_tensor("sel", [128, NPC], F32, kind="ExternalInput").ap()
    out = nc.dram_tensor("nll", [NPC, 1], F32, kind="ExternalOutput").ap()

    GCH = 4
    TCH = T // GCH  # t-steps per G chunk tile

    with tile.TileContext(nc) as tc:
        with (
            tc.tile_pool(name="persist", bufs=1) as persist,
            tc.tile_pool(name="stream", bufs=3) as stream,
            tc.tile_pool(name="scratch", bufs=2) as scratch,
            tc.tile_pool(name="psum", bufs=1, space="PSUM") as psum,
        ):
            g_ch = [
                persist.tile(
                    [NPC, TCH * LP], F32, tag=f"g_ch{c}", name=f"g_ch{c}"
                )
                for c in range(GCH)
            ]
            skip_sb = persist.tile([NPC, LP], F32)
            sel_sb = persist.tile([128, NPC], F32)
            zraw = persist.tile([128, NT], F32)
            zlog = persist.tile([128, NT], F32)
            zsum = persist.tile([128, 1], F32)
            snorm = persist.tile([NPC, NRN], F32)
            slog = persist.tile([NPC, NRN], F32)
            ssum = persist.tile([NPC, 1], F32)
            alpha_a = persist.tile([NPC, LP], F32, tag="alpha_a")
            alpha_b = persist.tile([NPC, LP], F32, tag="alpha_b")
            fir_out = persist.tile([NPC, LP], F32)
            rcp = persist.tile([NPC, 1], F32)
            fin = persist.tile([NPC, 1], F32)
            acc = persist.tile([NPC, 1], F32)
            nll_sb = persist.tile([NPC, 1], F32)
            zps = psum.tile([NPC, 1], F32)

            # Small inputs on the scalar-engine HWDGE queue: fast, and not
            # stuck behind the 2 MB stream DMAs on the sync queue. g_ch[0]
            # and skip come first — they gate the DP start.
            nc.scalar.dma_start(g_ch[0][:], g[:, : TCH * LP])
            nc.scalar.dma_start(skip_sb[:], skip)
            for c in range(1, GCH):
                nc.scalar.dma_start(
                    g_ch[c][:], g[:, c * TCH * LP : (c + 1) * TCH * LP]
                )
            nc.scalar.dma_start(sel_sb[:], sel)

            # G <- exp(G) in place, chunk by chunk (DP consumes in t-order).
            for c in range(GCH):
                nc.scalar.activation(g_ch[c][:], g_ch[c][:], AF.Exp)

            # Streaming softmax-normalizer pass. Stream tile partitions are
            # (t_inner, n) so each ACT accum gives Z for 16 t x 8 samples.
            for k in range(NT):
                xt = stream.tile([128, V], F32, tag="xt")
                src = x[:, k * TPB : (k + 1) * TPB, :].rearrange("n t v -> t n v")
                nc.sync.dma_start(xt[:], src)
                es = scratch.tile([128, V], F32, tag="es")
                nc.scalar.activation(
                    es[:], xt[:], AF.Exp, accum_out=zraw[:, k : k + 1]
                )

            # ---- CTC forward DP (2 DVE ops per step) ----
            nc.vector.memset(alpha_a[:], 0.0)
            nc.vector.memset(alpha_b[:], 0.0)
            # Flush the custom op's feedback flops with a zero input so no
            # stale NaN can leak through the first real call.
            nc.vector._custom_dve(
                fir3, out=fir_out[:], in0=alpha_b[:], in1=skip_sb[:]
            )
            # alpha_0 = G_0 at l=0,1 (cols 2:4 of the t=0 group).
            nc.vector.tensor_copy(alpha_a[:, 2:4], g_ch[0][:, 2:4])
            cur, nxt = alpha_a, alpha_b
            for t in range(1, T):
                gt = g_ch[t // TCH][:, (t % TCH) * LP : (t % TCH + 1) * LP]
                nc.vector._custom_dve(
                    fir3, out=fir_out[:], in0=cur[:], in1=skip_sb[:]
                )
                nc.vector.tensor_mul(nxt[:], fir_out[:], gt)
                if t % RENORM == RENORM - 1:
                    kk = t // RENORM
                    nc.vector.reduce_sum(snorm[:, kk : kk + 1], nxt[:], axis=AX.X)
                    nc.vector.reciprocal(rcp[:], snorm[:, kk : kk + 1])
                    nc.vector.tensor_scalar_mul(nxt[:], nxt[:], rcp[:])
                cur, nxt = nxt, cur

            # ---- epilogue ----
            # Keep the DVE instruction stream pure DP: all epilogue math
            # runs on ACT (Ln with fused accum), PE (partition-group sum),
            # and the otherwise-idle GPSIMD engine. A stray DVE op here
            # can be scheduled into the middle of the in-order DP stream
            # and head-of-line block it behind the streaming pass.
            nc.gpsimd.tensor_add(
                fin[:], cur[:, LP - 2 : LP - 1], cur[:, LP - 1 : LP]
            )
            nc.scalar.activation(zlog[:], zraw[:], AF.Ln, accum_out=zsum[:])
            nc.scalar.activation(slog[:], snorm[:], AF.Ln, accum_out=ssum[:])
            nc.scalar.activation(fin[:], fin[:], AF.Ln)
            # Partition-group sum of log Z: [8,1] = sel[128,8]^T @ zsum[128,1].
            nc.tensor.matmul(zps[:], lhsT=sel_sb[:], rhs=zsum[:], start=True, stop=True)
            # GPSIMD cannot read PSUM; bounce zps through ACT.
            zsb = persist.tile([NPC, 1], F32)
            nc.scalar.copy(zsb[:], zps[:])
            nc.gpsimd.tensor_add(acc[:], ssum[:], fin[:])
            nc.gpsimd.tensor_sub(nll_sb[:], zsb[:], acc[:])
            nc.gpsimd.dma_start(out, nll_sb[:])

    nc.compile()
    return nc


def _host_prep(y_pred, y_target):
    """Shard inputs and build the small derived tensors."""
    y_pred = np.ascontiguousarray(np.asarray(y_pred, dtype=np.float32))
    y_target = np.asarray(y_target, dtype=np.int32)

    ext = np.zeros((N, L), dtype=np.int64)
    ext[:, 1::2] = y_target
    # G[n, t, 2+s] = y_pred[n, t, ext[n, s]]; two lead columns hold -1e30
    # so exp() zeroes them (the DP guard re-zeroing trick).
    Gp = np.full((N, T, LP), NEGPAD, dtype=np.float32)
    Gp[:, :, 2:] = np.take_along_axis(y_pred, ext[:, None, :], axis=2)
    G = Gp.reshape(N, T * LP)

    # skip mask aligned with alpha columns: col 2+l <-> state l.
    skip01 = np.zeros((N, LP), dtype=np.float32)
    skip01[:, 5::2] = (y_target[:, 1:] != y_target[:, :-1]).astype(np.float32)

    sel = (np.arange(128)[:, None] % NPC == np.arange(NPC)[None, :]).astype(
        np.float32
    )

    in_maps = []
    for c in range(N_CORES):
        sl = slice(c * NPC, (c + 1) * NPC)
        in_maps.append(
            {
                "x": np.ascontiguousarray(y_pred[sl]),
                "g": np.ascontiguousarray(G[sl]),
                "skip": np.ascontiguousarray(skip01[sl]),
                "sel": sel,
            }
        )
    return in_maps


def _run(y_pred, y_target, trace=False):
    if "nc" not in _CACHE:
        _CACHE["nc"] = _build_program()
    nc = _CACHE["nc"]
    in_maps = _host_prep(y_pred, y_target)
    res = run_bass_kernel_spmd(
        nc, in_maps, core_ids=list(range(N_CORES)), trace=trace
    )
    nll = np.concatenate([r["nll"][:, 0] for r in res.results])
    loss = np.float32(np.mean(nll.astype(np.float64) / S))
    return np.asarray(loss, dtype=np.float32), res


def kernel(y_pred, y_target):
    loss, _ = _run(y_pred, y_target, trace=False)
    return loss


def kernel_traced(y_pred, y_target):
    """Like kernel() but with NTFF profiling; returns (loss, BassKernelResults)."""
    loss, res = _run(y_pred, y_target, trace=True)
    return loss, res

